# revision 1
# baseline (speedup 1.0000x reference)
"""Causal attention head kernel for Trainium2, 8 NeuronCores.

Problem: B=4, S=4096, D_IN=512, D_OUT=64, f32, causal, scale=1/sqrt(S).

Sharding: core c -> (batch b = c//2, k-shard hk = c%2). Each core handles ALL
queries of its batch but only the k-tiles (of 128 rows) with tile_index % 2 ==
hk, producing partial (numerator | denominator) sums; the host combines the
two k-shards. The instruction stream is identical across cores (SPMD):
causality differences between the two k-shards live in a small per-core mask
input (cmask2) and in the per-core gather of X_k/X_v rows. The host
pre-transposes X (layout prep) so the device loads d-on-partition tiles with
large contiguous DMAs.

Per-core device pipeline (f32 datapath, float32r matmuls: 1 cyc/row at N>=256):
  1. X^T tiles [128, 4(d-block), cols], one 1 MiB DMA per 512-col chunk,
     software-pipelined k-chunk/q-chunk schedule (k0 q7 q6 k1 q5 q4 ...).
  2. Projections: QT[64,4096], KT[64,2048] (head dim on partitions) and
     V_aug[128k, 65] = V | ones (ones column -> softmax denominator via the
     same PV matmul); V hi/lo split for k-tiles 0,1 cancels fp32r rounding
     where few-key rows can't average it out.
  3. Attention iterates k-chunk OUTER, q-chunk inner so exp work unlocks as
     soon as each k-chunk lands: S^T pair = KT_tile.T @ QT_chunk into a
     [128,1024] PSUM pair, one Exp activation per pair (PSUM -> SBUF f32r; no
     max subtraction -- scores are O(1) by construction: Q pre-scaled by
     1/sqrt(S)), diagonal-pair causal mask via elementwise mul, PV
     accumulation psum[65,512] per (q-chunk, k-chunk), DVE-accumulated into
     SBUF across k-chunks.
  4. Output PVT [65,4096] f32 per core; host: out[b] =
     ((PVT[2b] + PVT[2b+1])[0:64] / [64]).T
"""

import os

os.environ.setdefault("JAX_PLATFORMS", "cpu")

import numpy as np

import concourse.bass as bass
import concourse.bacc as bacc
import concourse.mybir as mybir
from concourse import tile
from concourse.bass_utils import run_bass_kernel_spmd

F32 = mybir.dt.float32
F32R = mybir.dt.float32r

B, S, D_IN, D_OUT = 4, 4096, 512, 64
SK = S // 2          # per-core k rows (interleaved 128-tiles)
N_KT = SK // 128     # 16 local k-tiles
N_QC = S // 512      # 8 q-chunks of 512
N_CORES = 8

_CACHE = {}


def mm(nc, out, lhsT, rhs, start, stop):
    nc.tensor.matmul(out, lhsT, rhs, start=start, stop=stop)


def build_nc():
    nc = bacc.Bacc(trn_type="TRN2", target_bir_lowering=False, debug=False)

    xqt_d = nc.dram_tensor("xqt", [D_IN, S], F32R, kind="ExternalInput").ap()
    xkt_d = nc.dram_tensor("xkt", [D_IN, SK], F32R, kind="ExternalInput").ap()
    xvt_d = nc.dram_tensor("xvt", [D_IN, SK], F32R, kind="ExternalInput").ap()
    wq = nc.dram_tensor("wq", [128, 4, D_OUT], F32R, kind="ExternalInput").ap()
    wk = nc.dram_tensor("wk", [128, 4, D_OUT], F32R, kind="ExternalInput").ap()
    wv = nc.dram_tensor("wv", [128, 4, D_OUT], F32R, kind="ExternalInput").ap()
    cm = nc.dram_tensor("cmask2", [128, 1024], F32R, kind="ExternalInput").ap()
    ones_d = nc.dram_tensor("ones16", [128, N_KT + 2], F32R, kind="ExternalInput").ap()
    pvt = nc.dram_tensor("pvt", [D_OUT + 1, S], F32, kind="ExternalOutput").ap()

    with tile.TileContext(nc) as tc:
        with (
            tc.tile_pool(name="persist", bufs=1) as pp,
            tc.tile_pool(name="et", bufs=3) as etp,
            tc.tile_pool(name="ostage", bufs=2) as osp,
            tc.tile_pool(name="ps_s", bufs=2, space="PSUM") as ps_s,
            tc.tile_pool(name="ps_pv", bufs=2, space="PSUM") as ps_pv,
            tc.tile_pool(name="ps_pr", bufs=2, space="PSUM") as ps_pr,
        ):
            # ---- persistent SBUF tiles ----
            # [128, 4(d-block), cols]: one 1 MiB DMA per 512-col chunk
            xqT = pp.tile([128, 4, S], F32R, tag="xqT", name="xqT")
            xkT = pp.tile([128, 4, SK], F32R, tag="xkT", name="xkT")
            xvT = pp.tile([128, 4, SK], F32R, tag="xvT", name="xvT")
            qt = pp.tile([64, S], F32R, tag="qt", name="qt")
            kt = pp.tile([64, SK], F32R, tag="kt", name="kt")
            vaug = pp.tile([128, N_KT, D_OUT + 1], F32R, tag="vaug", name="vaug")
            vaug_lo = pp.tile([128, 2, D_OUT + 1], F32R, tag="vaug_lo",
                              name="vaug_lo")
            cmask = pp.tile([128, 1024], F32R, tag="cmask", name="cmask")
            w_sb = {}
            for nm, src in (("wq", wq), ("wk", wk), ("wv", wv)):
                w_sb[nm] = pp.tile([128, 4, D_OUT], F32R, tag=nm, name=f"{nm}_sb")
                nc.sync.dma_start(out=w_sb[nm][:], in_=src[:])
            nc.sync.dma_start(out=cmask[:], in_=cm[:])
            # ones column for the softmax denominator
            nc.sync.dma_start(out=vaug[:, :, D_OUT], in_=ones_d[:, 0:N_KT])
            nc.sync.dma_start(out=vaug_lo[:, :, D_OUT], in_=ones_d[:, N_KT:])

            # ---- software-pipelined loads -> projections -> attention ----
            # Column-chunked loads (512 cols = 256 KiB per DMA) so q-chunk 0's
            # dependencies clear early and later loads overlap attention.
            def load_cols(dst_tile, src_ap, c, ncols):
                sl = slice(c * 512, (c + 1) * 512)
                src = src_ap.rearrange("(db p) c -> p db c", p=128)
                nc.sync.dma_start(out=dst_tile[:, :, sl], in_=src[:, :, sl])

            def proj_chunk(dst, xT, w, c, nm):
                sl = slice(c * 512, (c + 1) * 512)
                ps = ps_pr.tile([128, 512], F32, tag="ps_pr", name=f"pp_{nm}{c}")
                for dt in range(4):
                    mm(nc, ps[0:64, :], w[:, dt, :], xT[:, dt, sl],
                       start=(dt == 0), stop=(dt == 3))
                nc.vector.tensor_copy(dst[:, sl], ps[0:64, :])

            def v_chunk(c):
                # V natural for s-tiles 4c..4c+3: [128s,64] per tile
                for t in range(4 * c, 4 * c + 4):
                    ps = ps_pr.tile([128, 512], F32, tag="ps_pr", name=f"pv_{t}")
                    for dt in range(4):
                        mm(nc, ps[:, 0:D_OUT],
                           xvT[:, dt, t * 128:(t + 1) * 128],
                           w_sb["wv"][:, dt, :],
                           start=(dt == 0), stop=(dt == 3))
                    nc.vector.tensor_copy(vaug[:, t, 0:D_OUT], ps[:, 0:D_OUT])
                    if t < 2:  # hi/lo split: lo = exact - rounded(hi)
                        nc.vector.tensor_sub(
                            vaug_lo[:, t, 0:D_OUT], ps[:, 0:D_OUT],
                            vaug[:, t, 0:D_OUT])

            # SBUF accumulators for the k-chunk-outer PV partial sums
            acc = [pp.tile([65, 512], F32, tag=f"acc{j}", name=f"acc{j}")
                   for j in range(N_QC)]

            def kv_stage(kc):
                load_cols(xkT, xkt_d, kc, SK)
                load_cols(xvT, xvt_d, kc, SK)
                proj_chunk(kt, xkT, w_sb["wk"], kc, "k")
                v_chunk(kc)

            def q_stage(j):
                load_cols(xqT, xqt_d, j, S)
                proj_chunk(qt, xqT, w_sb["wq"], j, "q")

            def attn_block(j, kc):
                # pairs i of q-chunk j whose k-tiles (2i, 2i+1) lie in k-chunk
                # kc; i == j is the diagonal (masked) pair.
                iis = [i for i in (2 * kc, 2 * kc + 1) if i <= j]
                if not iis:
                    return
                qs = qt[:, j * 512:(j + 1) * 512]
                pv = ps_pv.tile([65, 512], F32, tag="ps_pv", name=f"pvp{j}_{kc}")
                pv_mms = []  # (lhsT, rhs) accumulation group, flags at end
                ets = {}
                for n, i in enumerate(iis):
                    ps = ps_s.tile([128, 1024], F32, tag="ps_s", name=f"st{j}_{i}")
                    for h in range(2):
                        t = 2 * i + h
                        mm(nc, ps[:, h * 512:(h + 1) * 512],
                           kt[:, t * 128:(t + 1) * 128], qs,
                           start=True, stop=True)
                    et = etp.tile([128, 1024], F32R, tag="et", name=f"et{j}_{i}")
                    nc.scalar.activation(
                        et[:], ps[:], mybir.ActivationFunctionType.Exp)
                    if i == j:  # diagonal pair: causal mask
                        nc.vector.tensor_mul(et[:], et[:], cmask[:])
                    for h in range(2):
                        t = 2 * i + h
                        eh = et[:, h * 512:(h + 1) * 512]
                        pv_mms.append((vaug[:, t, :], eh))
                        if i == 0:
                            pv_mms.append((vaug_lo[:, t, :], eh))
                for n, (lh, rh) in enumerate(pv_mms):
                    mm(nc, pv[:], lh, rh,
                       start=(n == 0), stop=(n == len(pv_mms) - 1))
                if kc == 0:
                    nc.vector.tensor_copy(acc[j][:], pv[:])
                else:
                    nc.vector.tensor_add(acc[j][:], acc[j][:], pv[:])
                if kc == j // 2:  # last k-chunk for this q-chunk: emit output
                    nc.sync.dma_start(
                        out=pvt[:, j * 512:(j + 1) * 512], in_=acc[j][:])

            # Loads ordered so exp work unlocks steadily: k0/v0 first, then
            # q7..q0 (heavy chunks early) with k1..k3 interleaved. An
            # attn_block(j, kc) may only be emitted after BOTH q_stage(j) and
            # kv_stage(kc) (program-order read-after-write).
            schedule = ["k0", "q7", "q6", "k1", "q5", "q4", "k2",
                        "q3", "q2", "q1", "k3", "q0"]
            done_q, done_k = set(), set()
            for item in schedule:
                if item[0] == "k":
                    kc = int(item[1])
                    kv_stage(kc)
                    done_k.add(kc)
                    for j in sorted(done_q, reverse=True):
                        attn_block(j, kc)
                else:
                    j = int(item[1])
                    q_stage(j)
                    done_q.add(j)
                    for kc in sorted(done_k):
                        attn_block(j, kc)
    nc.compile()
    return nc


def _prep_w(w, scale=1.0):
    # [512, 64] -> [128, 4, 64]: (p, dt, e) holds W[dt*128 + p, e] so the
    # lhsT slice [:, dt, :] matches X^T d-block dt.
    return np.ascontiguousarray(
        (w * scale).reshape(4, 128, D_OUT).transpose(1, 0, 2).astype(np.float32))


def kernel(inputs_for_keys, inputs_for_values, inputs_for_queries, WK, WV, WQ):
    xk_f = np.asarray(inputs_for_keys, np.float32)
    xv_f = np.asarray(inputs_for_values, np.float32)
    xq_f = np.asarray(inputs_for_queries, np.float32)
    wkp = _prep_w(np.asarray(WK, np.float32))
    wvp = _prep_w(np.asarray(WV, np.float32))
    wqp = _prep_w(np.asarray(WQ, np.float32), scale=1.0 / np.sqrt(np.float32(S)))

    if "nc" not in _CACHE:
        _CACHE["nc"] = build_nc()
    nc = _CACHE["nc"]

    # cmask2[k, 0:512]   : pair member h=0 -> 1 if q >= k + 128*hk
    # cmask2[k, 512:1024]: pair member h=1 -> 1 if q >= k + 256 + 128*hk
    kk = np.arange(128)[:, None]
    qq = np.arange(512)[None, :]
    cms = []
    for hk in range(2):
        m0 = (qq >= kk + 128 * hk).astype(np.float32)
        m1 = (qq >= kk + 256 + 128 * hk).astype(np.float32)
        cms.append(np.ascontiguousarray(np.concatenate([m0, m1], axis=1)))

    ones16 = np.concatenate([np.ones((128, N_KT), np.float32),
                         np.zeros((128, 2), np.float32)], axis=1)
    in_maps = []
    xqt_b = [np.ascontiguousarray(xq_f[b].T) for b in range(B)]
    for c in range(N_CORES):
        b, hk = c // 2, c % 2
        xk_g = xk_f[b].reshape(S // 128, 128, D_IN)[hk::2].reshape(SK, D_IN)
        xv_g = xv_f[b].reshape(S // 128, 128, D_IN)[hk::2].reshape(SK, D_IN)
        in_maps.append({
            "xqt": xqt_b[b],
            "xkt": np.ascontiguousarray(xk_g.T),
            "xvt": np.ascontiguousarray(xv_g.T),
            "wq": wqp, "wk": wkp, "wv": wvp,
            "cmask2": cms[hk],
            "ones16": ones16,
        })

    _CACHE["in_maps"] = in_maps
    res = run_bass_kernel_spmd(nc, in_maps, core_ids=list(range(N_CORES)))
    out = np.empty((B, S, D_OUT), np.float32)
    for b in range(B):
        p = res.results[2 * b]["pvt"] + res.results[2 * b + 1]["pvt"]
        out[b] = (p[0:D_OUT, :] / p[D_OUT:D_OUT + 1, :]).T
    return out



# revision 7
# speedup vs baseline: 1.4386x; 1.4386x over previous
"""Causal attention head kernel for Trainium2, 8 NeuronCores — v2 (fp16).

Problem: B=4, S=4096, D_IN=512, D_OUT=64, f32, causal, scale=1/sqrt(S).

Sharding: core c -> (batch b = c//2, k-shard hk = c%2). Each core handles ALL
queries of its batch but only the k-tiles (of 128 rows) with tile_index % 2 ==
hk, producing partial (numerator | denominator) sums; the host combines the
two k-shards. SPMD: causality differences between the two k-shards live in a
per-core mask input and in the per-core gather of X_k/X_v rows.

v2 datapath (vs the f32r baseline):
  * fp16 X/W/Q/K/V/P everywhere except q-chunk 0 (rows 0-511), which runs the
    baseline-style f32r path (few-key rows can't average out fp16 noise).
    fp16 halves DMA and keeps 1 cyc/row matmuls.
  * PV uses the transposed-P orientation: out[q,e] = sum_k P^T[k,q].T V[k,e]
    with lhsT = exp-tile slice [128k,128q], rhs = V_aug [128k,65] -> M=128
    (full PE) instead of M=65: PV cost 65 rows per (q-tile, k-tile) vs 512.
  * Flash-style q-chunk-outer schedule; per-chunk PV accumulators live in one
    PSUM bank as a single accumulation group (start zeroes the 2KB zero
    region, later matmuls first-touch-overwrite their slots).
  * Diagonal pair trimmed to 768 cols (the h=1 half-tile only covers q-cols
    256:512; cols 0:256 are causally dead for both shards).
  * V_aug = V | ones column -> softmax denominator via the same PV matmuls.
  * DMA issue is dual-lane (SP via HWDGE + GPSIMD via SWDGE) with k|v and
    wq|wk|wv merged into single dram tensors - the 650ns/DMA issue rate on a
    single queue was gating startup.  Output DMAs issue from DVE right after
    the staging copy (no cross-engine sem hop).
  * PE warmup matmuls hold the p-state ramp while the first loads land.

Output: corner pvt0 [65,512] (chunk 0, transposed) + pvtl [7*128,260]
(chunks 1-7, q-on-partition); host assembles [4096,65] per core, sums the two
k-shard partials, and divides.
"""

import os

os.environ.setdefault("JAX_PLATFORMS", "cpu")

import numpy as np

import concourse.bass as bass
import concourse.bacc as bacc
import concourse.mybir as mybir
from concourse import tile
from concourse.bass_utils import run_bass_kernel_spmd

F32 = mybir.dt.float32
F32R = mybir.dt.float32r
F16 = mybir.dt.float16

B, S, D_IN, D_OUT = 4, 4096, 512, 64
SK = S // 2          # per-core k rows (interleaved 128-tiles)
N_KT = SK // 128     # 16 local k-tiles
N_QC = S // 512      # 8 q-chunks of 512
N_CORES = 8
EXP = mybir.ActivationFunctionType.Exp

_CACHE = {}


def mm(nc, out, lhsT, rhs, start, stop):
    nc.tensor.matmul(out, lhsT, rhs, start=start, stop=stop)


def build_nc():
    nc = bacc.Bacc(trn_type="TRN2", target_bir_lowering=False, debug=False)

    # corner (q-chunk 0 / k-tiles 0,1) f32r inputs
    xq_r = nc.dram_tensor("xq_r", [128, 4, 512], F32R, kind="ExternalInput").ap()
    xkv_r = nc.dram_tensor("xkv_r", [128, 8, 256], F32R, kind="ExternalInput").ap()
    w_r = nc.dram_tensor("w_r", [128, 12, D_OUT], F32R, kind="ExternalInput").ap()
    cm_r = nc.dram_tensor("cm_r", [128, 768], F32R, kind="ExternalInput").ap()
    # fp16 bulk inputs ([q|k|v] packed along the d-block dim where possible)
    xqh = nc.dram_tensor("xqh", [128, 4, 3584], F16, kind="ExternalInput").ap()
    xkvh = nc.dram_tensor("xkvh", [128, 8, SK], F16, kind="ExternalInput").ap()
    w_h = nc.dram_tensor("w_h", [128, 12, D_OUT], F16, kind="ExternalInput").ap()
    cm_h = nc.dram_tensor("cm_h", [128, 768], F16, kind="ExternalInput").ap()
    # outputs: partial (numerator | denominator)
    pvt0 = nc.dram_tensor("pvt0", [D_OUT + 1, 512], F32, kind="ExternalOutput").ap()
    pvtl = nc.dram_tensor("pvtl", [7 * 128, 260], F32, kind="ExternalOutput").ap()

    with tile.TileContext(nc) as tc:
        with (
            tc.tile_pool(name="persist", bufs=1) as pp,
            tc.tile_pool(name="et", bufs=4) as etp,
            tc.tile_pool(name="ostage", bufs=6) as osp,
            tc.tile_pool(name="ps_s", bufs=2, space="PSUM") as ps_s,
            tc.tile_pool(name="ps_pv", bufs=2, space="PSUM") as ps_pv,
            tc.tile_pool(name="ps_pr", bufs=2, space="PSUM") as ps_pr,
        ):
            # ---- persistent SBUF tiles ----
            xqTr = pp.tile([128, 4, 512], F32R, tag="xqTr", name="xqTr")
            xkvTr = pp.tile([128, 8, 256], F32R, tag="xkvTr", name="xkvTr")
            xqT = pp.tile([128, 4, 3584], F16, tag="xqT", name="xqT")
            xkvT = pp.tile([128, 8, SK], F16, tag="xkvT", name="xkvT")
            qtr = pp.tile([64, 512], F32R, tag="qtr", name="qtr")
            ktr = pp.tile([64, 256], F32R, tag="ktr", name="ktr")
            qt = pp.tile([64, 3584], F16, tag="qt", name="qt")
            kt = pp.tile([64, SK], F16, tag="kt", name="kt")
            vaug = pp.tile([128, N_KT, D_OUT + 2], F16, tag="vaug", name="vaug")
            vhir = pp.tile([128, 2, D_OUT + 1], F32R, tag="vhir", name="vhir")
            vlor = pp.tile([128, 2, D_OUT + 1], F32R, tag="vlor", name="vlor")
            cmask_r = pp.tile([128, 768], F32R, tag="cmask_r", name="cmask_r")
            cmask_h = pp.tile([128, 768], F16, tag="cmask_h", name="cmask_h")
            etr = pp.tile([128, 768], F32R, tag="etr", name="etr")
            oc = pp.tile([D_OUT + 1, 512], F32, tag="oc", name="oc")
            wsb_h = pp.tile([128, 12, D_OUT], F16, tag="wsb_h", name="wsb_h")
            wsb_r = pp.tile([128, 12, D_OUT], F32R, tag="wsb_r", name="wsb_r")
            wq_h, wk_h, wv_h = (wsb_h[:, 0:4, :], wsb_h[:, 4:8, :],
                                wsb_h[:, 8:12, :])
            wq_rs, wk_rs, wv_rs = (wsb_r[:, 0:4, :], wsb_r[:, 4:8, :],
                                   wsb_r[:, 8:12, :])

            # PE warmup: dummy matmuls with no DMA dependency keep the PE
            # busy (and its p-state ramping) while the first loads land.
            wu = pp.tile([128, 576], F16, tag="wu", name="wu")
            nc.vector.memset(wu[:].opt().bitcast(mybir.dt.uint32), 0)
            for n in range(14):
                ps = ps_pr.tile([128, 512], F32, tag="ps_pr", name=f"wu{n}")
                mm(nc, ps[0:64, :], wu[:, 0:64], wu[:, 64:576],
                   start=True, stop=True)

            # ones columns for the softmax denominator (V copies overwrite
            # cols 0:64 of each slot; col 64 stays at the memset value)
            nc.vector.memset(vaug[:].opt().bitcast(mybir.dt.uint32),
                             0x3C003C00)  # fp16 1.0 pair
            nc.vector.memset(vhir[:].opt().bitcast(mybir.dt.uint32),
                             0x3F800000)  # f32 1.0
            nc.vector.memset(vlor[:].opt().bitcast(mybir.dt.uint32), 0)

            def load_q(lane, c0, c1):
                lane.dma_start(out=xqT[:, :, c0:c1], in_=xqh[:, :, c0:c1])

            def load_kv(lane, p):
                sl = slice(p * 256, (p + 1) * 256)
                lane.dma_start(out=xkvT[:, :, sl], in_=xkvh[:, :, sl])

            # Dual-lane loads in compute order.  SP issues through HWDGE,
            # gpsimd through SWDGE - the two DGE paths run concurrently.
            sp, gp = nc.sync, nc.gpsimd
            gp.dma_start(out=cmask_h[:], in_=cm_h[:])
            sp.dma_start(out=wsb_h[:], in_=w_h[:])
            load_q(sp, 0, 512)                     # q1
            # k-part of pair 0 first: it gates the first S matmul
            sp.dma_start(out=xkvT[:, 0:4, 0:256], in_=xkvh[:, 0:4, 0:256])
            sp.dma_start(out=xkvT[:, 4:8, 0:256], in_=xkvh[:, 4:8, 0:256])
            load_kv(sp, 1)
            load_q(sp, 512, 1024)                  # q2
            load_kv(sp, 2)
            load_q(sp, 1024, 1536)                 # q3
            load_kv(sp, 3)
            load_q(sp, 1536, 2048)                 # q4
            load_kv(sp, 4)
            load_q(sp, 2048, 2560)                 # q5
            load_kv(sp, 5)
            load_q(sp, 2560, 3072)                 # q6
            load_kv(sp, 6)
            # (compute order: q_stage(j) precedes the k/v stages it overlaps)
            sp.dma_start(out=wsb_r[:], in_=w_r[:])
            sp.dma_start(out=xkvTr[:], in_=xkv_r[:])
            nc.vector.tensor_copy(cmask_r[:], cmask_h[:])
            sp.dma_start(out=xqTr[:], in_=xq_r[:])
            load_kv(sp, 7)
            load_q(sp, 3072, 3584)                 # q7

            def corner():
                ps = ps_pr.tile([128, 512], F32, tag="ps_pr", name="pp_kr")
                for dt in range(4):
                    mm(nc, ps[0:64, 0:256], wk_rs[:, dt, :],
                       xkvTr[:, dt, :], start=(dt == 0), stop=(dt == 3))
                nc.vector.tensor_copy(ktr[:], ps[0:64, 0:256])
                for l in range(2):
                    ps = ps_pr.tile([128, 512], F32, tag="ps_pr",
                                    name=f"pp_vr{l}")
                    for dt in range(4):
                        mm(nc, ps[:, 0:D_OUT],
                           xkvTr[:, 4 + dt, l * 128:(l + 1) * 128],
                           wv_rs[:, dt, :],
                           start=(dt == 0), stop=(dt == 3))
                    nc.vector.tensor_copy(vhir[:, l, 0:D_OUT], ps[:, 0:D_OUT])
                    nc.vector.tensor_sub(
                        vlor[:, l, 0:D_OUT], ps[:, 0:D_OUT],
                        vhir[:, l, 0:D_OUT])
                ps = ps_pr.tile([128, 512], F32, tag="ps_pr", name="pp_qr")
                for dt in range(4):
                    mm(nc, ps[0:64, :], wq_rs[:, dt, :], xqTr[:, dt, :],
                       start=(dt == 0), stop=(dt == 3))
                nc.vector.tensor_copy(qtr[:], ps[0:64, :])

                ps = ps_s.tile([128, 1024], F32, tag="ps_s", name="st_r")
                mm(nc, ps[:, 0:512], ktr[:, 0:128], qtr[:],
                   start=True, stop=True)
                mm(nc, ps[:, 512:768], ktr[:, 128:256], qtr[:, 256:512],
                   start=True, stop=True)
                nc.scalar.activation(etr[:], ps[:, 0:768], EXP)
                nc.vector.tensor_mul(etr[:], etr[:], cmask_r[:])
                pv = ps_pv.tile([128, 512], F32, tag="ps_pv", name="pv_r")
                mm(nc, pv[0:65, :], vhir[:, 0, :], etr[:, 0:512],
                   start=True, stop=False)
                mm(nc, pv[0:65, :], vlor[:, 0, :], etr[:, 0:512],
                   start=False, stop=False)
                mm(nc, pv[0:65, 256:512], vhir[:, 1, :], etr[:, 512:768],
                   start=False, stop=False)
                mm(nc, pv[0:65, 256:512], vlor[:, 1, :], etr[:, 512:768],
                   start=False, stop=True)
                nc.vector.tensor_copy(oc[:], pv[0:65, :])
                nc.sync.dma_start(out=pvt0[:], in_=oc[:])

            # ---------------- fp16 bulk ----------------
            def kproj(p):
                # k-pair p: tiles 2p, 2p+1 (cols 256p : 256p+256)
                sl = slice(p * 256, (p + 1) * 256)
                ps = ps_pr.tile([128, 512], F32, tag="ps_pr", name=f"pp_k{p}")
                for dt in range(4):
                    mm(nc, ps[0:64, 0:256], wk_h[:, dt, :],
                       xkvT[:, dt, sl], start=(dt == 0), stop=(dt == 3))
                nc.vector.tensor_copy(kt[:, sl], ps[0:64, 0:256])

            def vproj(p):
                for l in (2 * p, 2 * p + 1):
                    ps = ps_pr.tile([128, 512], F32, tag="ps_pr", name=f"pp_v{l}")
                    for dt in range(4):
                        mm(nc, ps[:, 0:D_OUT],
                           xkvT[:, 4 + dt, l * 128:(l + 1) * 128],
                           wv_h[:, dt, :],
                           start=(dt == 0), stop=(dt == 3))
                    nc.vector.tensor_copy(vaug[:, l, 0:D_OUT], ps[:, 0:D_OUT])

            def q_stage(j):
                sl = slice((j - 1) * 512, j * 512)
                ps = ps_pr.tile([128, 512], F32, tag="ps_pr", name=f"pp_q{j}")
                for dt in range(4):
                    mm(nc, ps[0:64, :], wq_h[:, dt, :], xqT[:, dt, sl],
                       start=(dt == 0), stop=(dt == 3))
                nc.vector.tensor_copy(qt[:, sl], ps[0:64, :])

            def chunk(j, diag_first=False, pre=()):
                qs = qt[:, (j - 1) * 512:j * 512]
                pv = ps_pv.tile([128, 512], F32, tag="ps_pv", name=f"pv{j}")
                ets = {}
                # diag-first shortens the exp->mask->PV->copy tail of the
                # final chunk (its last processed pair needs no mask)
                order = [j] + list(range(j)) if diag_first else \
                    list(range(j + 1))

                def s_pair(i):
                    ps = ps_s.tile([128, 1024], F32, tag="ps_s",
                                   name=f"st{j}_{i}")
                    mm(nc, ps[:, 0:512], kt[:, (2 * i) * 128:(2 * i + 1) * 128],
                       qs, start=True, stop=True)
                    if i < j:
                        mm(nc, ps[:, 512:1024],
                           kt[:, (2 * i + 1) * 128:(2 * i + 2) * 128], qs,
                           start=True, stop=True)
                        et = etp.tile([128, 1024], F16, tag="et",
                                      name=f"et{j}_{i}")
                        nc.scalar.activation(et[:], ps[:], EXP)
                    else:
                        mm(nc, ps[:, 512:768],
                           kt[:, (2 * i + 1) * 128:(2 * i + 2) * 128],
                           qs[:, 256:512], start=True, stop=True)
                        et = etp.tile([128, 1024], F16, tag="et",
                                      name=f"et{j}_{i}")
                        nc.scalar.activation(et[:, 0:768], ps[:, 0:768], EXP)
                        nc.vector.tensor_mul(
                            et[:, 0:768], et[:, 0:768], cmask_h[:])
                    ets[i] = et

                def pv_pair(i, first, last):
                    et = ets[i]
                    mms = []
                    for t in range(4):
                        for h in range(2):
                            if i == j and h == 1 and t < 2:
                                continue
                            l = 2 * i + h
                            if h == 0:
                                esl = slice(128 * t, 128 * t + 128)
                            elif i < j:
                                esl = slice(512 + 128 * t, 512 + 128 * t + 128)
                            else:
                                esl = slice(512 + 128 * (t - 2),
                                            512 + 128 * (t - 2) + 128)
                            mms.append((t, et[:, esl], vaug[:, l, 0:D_OUT + 1]))
                    for n, (t, lh, rh) in enumerate(mms):
                        mm(nc, pv[:, t * 128:t * 128 + 65], lh, rh,
                           start=(first and n == 0),
                           stop=(last and n == len(mms) - 1))

                for n, i in enumerate(order):
                    s_pair(i)
                    if n == 0:
                        for f in pre:
                            f()
                    if n > 0:
                        pv_pair(order[n - 1], first=(n == 1), last=False)
                pv_pair(order[-1], first=(j == 0), last=True)
                ost = osp.tile([128, 260], F32, tag="ost", name=f"ost{j}")
                src = pv.rearrange("p (t c) -> p t c", t=4)
                dst = ost.rearrange("p (t c) -> p t c", t=4, c=65)
                nc.vector.tensor_copy(dst[:], src[:, :, 0:65])
                lane = nc.sync if j == 7 else nc.gpsimd
                lane.dma_start(
                    out=pvtl[(j - 1) * 128:j * 128, :], in_=ost[:])

            import functools
            q_stage(1)
            kproj(0)
            chunk(1, pre=(functools.partial(vproj, 0),
                          functools.partial(kproj, 1),
                          functools.partial(vproj, 1)))
            for j in range(2, 7):
                q_stage(j)
                chunk(j, pre=(functools.partial(kproj, j),
                              functools.partial(vproj, j)))
            corner()
            kproj(7)
            vproj(7)
            q_stage(7)
            chunk(7, diag_first=True)
    nc.compile()
    return nc


def _prep_w(w, scale=1.0):
    # [512, 64] -> [128, 4, 64]: (p, dt, e) holds W[dt*128 + p, e]
    return np.ascontiguousarray(
        (w * scale).reshape(4, 128, D_OUT).transpose(1, 0, 2).astype(np.float32))


def _dblock(xT):
    # [512, C] -> [128, 4, C]
    return np.ascontiguousarray(
        xT.reshape(4, 128, -1).transpose(1, 0, 2))


def kernel(inputs_for_keys, inputs_for_values, inputs_for_queries, WK, WV, WQ):
    xk_f = np.asarray(inputs_for_keys, np.float32)
    xv_f = np.asarray(inputs_for_values, np.float32)
    xq_f = np.asarray(inputs_for_queries, np.float32)
    wkp = _prep_w(np.asarray(WK, np.float32))
    wvp = _prep_w(np.asarray(WV, np.float32))
    wqp = _prep_w(np.asarray(WQ, np.float32), scale=1.0 / np.sqrt(np.float32(S)))
    wcat = np.concatenate([wqp, wkp, wvp], axis=1)  # [128, 12, 64]

    if "nc" not in _CACHE:
        _CACHE["nc"] = build_nc()
    nc = _CACHE["nc"]

    # cmask[p, c]       (c in 0:512):  1 if c >= p + 128*hk      (diag h=0)
    # cmask[p, 512+cc]  (cc in 0:256): 1 if cc >= p + 128*hk     (diag h=1)
    kk = np.arange(128)[:, None]
    cc512 = np.arange(512)[None, :]
    cms = []
    for hk in range(2):
        m0 = (cc512 >= kk + 128 * hk).astype(np.float32)
        cms.append(np.ascontiguousarray(
            np.concatenate([m0, m0[:, 0:256]], axis=1)))

    in_maps = []
    for c in range(N_CORES):
        b, hk = c // 2, c % 2
        xk_g = xk_f[b].reshape(S // 128, 128, D_IN)[hk::2].reshape(SK, D_IN)
        xv_g = xv_f[b].reshape(S // 128, 128, D_IN)[hk::2].reshape(SK, D_IN)
        xq_db = _dblock(xq_f[b].T)
        xkv_db = np.concatenate([_dblock(xk_g.T), _dblock(xv_g.T)], axis=1)
        in_maps.append({
            "xq_r": np.ascontiguousarray(xq_db[:, :, 0:512]),
            "xkv_r": np.ascontiguousarray(xkv_db[:, :, 0:256]),
            "xqh": np.ascontiguousarray(xq_db[:, :, 512:]).astype(np.float16),
            "xkvh": xkv_db.astype(np.float16),
            "w_r": wcat,
            "w_h": wcat.astype(np.float16),
            "cm_r": cms[hk],
            "cm_h": cms[hk].astype(np.float16),
        })

    _CACHE["in_maps"] = in_maps
    res = run_bass_kernel_spmd(nc, in_maps, core_ids=list(range(N_CORES)))
    out = np.empty((B, S, D_OUT), np.float32)
    for b in range(B):
        full = np.empty((S, D_OUT + 1), np.float32)
        for kshard in range(2):
            r = res.results[2 * b + kshard]
            part = np.empty((S, D_OUT + 1), np.float32)
            part[0:512] = r["pvt0"].T
            part[512:] = (r["pvtl"].reshape(7, 128, 4, 65)
                          .transpose(0, 2, 1, 3).reshape(3584, 65))
            if kshard == 0:
                full[:] = part
            else:
                full += part
        out[b] = full[:, 0:D_OUT] / full[:, D_OUT:D_OUT + 1]
    return out


# revision 16
# speedup vs baseline: 1.4407x; 1.0015x over previous
"""Causal attention head kernel for Trainium2, 8 NeuronCores — v2 (fp16).

Problem: B=4, S=4096, D_IN=512, D_OUT=64, f32, causal, scale=1/sqrt(S).

Sharding: core c -> (batch b = c//2, k-shard hk = c%2). Each core handles ALL
queries of its batch but only the k-tiles (of 128 rows) with tile_index % 2 ==
hk, producing partial (numerator | denominator) sums; the host combines the
two k-shards. SPMD: causality differences between the two k-shards live in a
per-core mask input and in the per-core gather of X_k/X_v rows.

v2 datapath (vs the f32r baseline):
  * fp16 X/W/Q/K/V/P everywhere except q-chunk 0 (rows 0-511), which runs the
    baseline-style f32r path (few-key rows can't average out fp16 noise).
    fp16 halves DMA and keeps 1 cyc/row matmuls.
  * PV uses the transposed-P orientation: out[q,e] = sum_k P^T[k,q].T V[k,e]
    with lhsT = exp-tile slice [128k,128q], rhs = V_aug [128k,65] -> M=128
    (full PE) instead of M=65: PV cost 65 rows per (q-tile, k-tile) vs 512.
  * Flash-style q-chunk-outer schedule; per-chunk PV accumulators live in one
    PSUM bank as a single accumulation group (start zeroes the 2KB zero
    region, later matmuls first-touch-overwrite their slots).
  * Diagonal pair trimmed to 768 cols (the h=1 half-tile only covers q-cols
    256:512; cols 0:256 are causally dead for both shards).
  * V_aug = V | ones column -> softmax denominator via the same PV matmuls.
  * Inputs stream on the SP/HWDGE lane in exact compute-need order (k|v and
    wq|wk|wv merged into single dram tensors to cut the ~650ns/DMA issue
    cost); the DMA engines then run back-to-back at the 360GB/s model
    roofline.  Corner f32r tensors load late (needed only after chunk 6).
    Output DMAs issue from GPSIMD/SWDGE (chunk 7 + corner from SP) so they
    never stall the input issue stream; the output staging pool is 7 deep
    because output transfers queue behind the saturated input stream.
  * PE warmup matmuls hold the p-state ramp while the first loads land.
  * Exp runs only on the Activation engine (~37us busy = the critical
    engine); kproj/vproj/q_stage are interleaved into chunk pair-loops via
    hooks so PE keeps ~1 S-pair ahead of Act.

Output: corner pvt0 [65,512] (chunk 0, transposed) + pvtl [7*128,260]
(chunks 1-7, q-on-partition); host assembles [4096,65] per core, sums the two
k-shard partials, and divides.
"""

import os

os.environ.setdefault("JAX_PLATFORMS", "cpu")

import numpy as np

import concourse.bass as bass
import concourse.bacc as bacc
import concourse.mybir as mybir
from concourse import tile
from concourse.bass_utils import run_bass_kernel_spmd

F32 = mybir.dt.float32
F32R = mybir.dt.float32r
F16 = mybir.dt.float16

B, S, D_IN, D_OUT = 4, 4096, 512, 64
SK = S // 2          # per-core k rows (interleaved 128-tiles)
N_KT = SK // 128     # 16 local k-tiles
N_QC = S // 512      # 8 q-chunks of 512
N_CORES = 8
EXP = mybir.ActivationFunctionType.Exp

_CACHE = {}


def mm(nc, out, lhsT, rhs, start, stop):
    nc.tensor.matmul(out, lhsT, rhs, start=start, stop=stop)


def build_nc():
    nc = bacc.Bacc(trn_type="TRN2", target_bir_lowering=False, debug=False)

    # corner (q-chunk 0 / k-tiles 0,1) f32r inputs
    xq_r = nc.dram_tensor("xq_r", [128, 4, 512], F32R, kind="ExternalInput").ap()
    xkv_r = nc.dram_tensor("xkv_r", [128, 8, 256], F32R, kind="ExternalInput").ap()
    w_r = nc.dram_tensor("w_r", [128, 12, D_OUT], F32R, kind="ExternalInput").ap()
    cm_r = nc.dram_tensor("cm_r", [128, 768], F32R, kind="ExternalInput").ap()
    # fp16 bulk inputs ([q|k|v] packed along the d-block dim where possible)
    xqh = nc.dram_tensor("xqh", [128, 4, 3584], F16, kind="ExternalInput").ap()
    xkvh = nc.dram_tensor("xkvh", [128, 8, SK], F16, kind="ExternalInput").ap()
    w_h = nc.dram_tensor("w_h", [128, 12, D_OUT], F16, kind="ExternalInput").ap()
    cm_h = nc.dram_tensor("cm_h", [128, 768], F16, kind="ExternalInput").ap()
    # outputs: partial (numerator | denominator)
    pvt0 = nc.dram_tensor("pvt0", [D_OUT + 1, 512], F32, kind="ExternalOutput").ap()
    pvtl = nc.dram_tensor("pvtl", [7 * 128, 260], F32, kind="ExternalOutput").ap()

    with tile.TileContext(nc) as tc:
        with (
            tc.tile_pool(name="persist", bufs=1) as pp,
            tc.tile_pool(name="et", bufs=6) as etp,
            tc.tile_pool(name="ostage", bufs=7) as osp,
            tc.tile_pool(name="ps_s", bufs=2, space="PSUM") as ps_s,
            tc.tile_pool(name="ps_pv", bufs=2, space="PSUM") as ps_pv,
            tc.tile_pool(name="ps_pr", bufs=2, space="PSUM") as ps_pr,
        ):
            # ---- persistent SBUF tiles ----
            xqTr = pp.tile([128, 4, 512], F32R, tag="xqTr", name="xqTr")
            xkvTr = pp.tile([128, 8, 256], F32R, tag="xkvTr", name="xkvTr")
            xqT = pp.tile([128, 4, 3584], F16, tag="xqT", name="xqT")
            xkvT = pp.tile([128, 8, SK], F16, tag="xkvT", name="xkvT")
            qtr = pp.tile([64, 512], F32R, tag="qtr", name="qtr")
            ktr = pp.tile([64, 256], F32R, tag="ktr", name="ktr")
            qt = pp.tile([64, 3584], F16, tag="qt", name="qt")
            kt = pp.tile([64, SK], F16, tag="kt", name="kt")
            vaug = pp.tile([128, N_KT, D_OUT + 2], F16, tag="vaug", name="vaug")
            vhir = pp.tile([128, 2, D_OUT + 1], F32R, tag="vhir", name="vhir")
            vlor = pp.tile([128, 2, D_OUT + 1], F32R, tag="vlor", name="vlor")
            cmask_r = pp.tile([128, 768], F32R, tag="cmask_r", name="cmask_r")
            cmask_h = pp.tile([128, 768], F16, tag="cmask_h", name="cmask_h")
            etr = pp.tile([128, 768], F32R, tag="etr", name="etr")
            oc = pp.tile([D_OUT + 1, 512], F32, tag="oc", name="oc")
            wsb_h = pp.tile([128, 12, D_OUT], F16, tag="wsb_h", name="wsb_h")
            wsb_r = pp.tile([128, 12, D_OUT], F32R, tag="wsb_r", name="wsb_r")
            wq_h, wk_h, wv_h = (wsb_h[:, 0:4, :], wsb_h[:, 4:8, :],
                                wsb_h[:, 8:12, :])
            wq_rs, wk_rs, wv_rs = (wsb_r[:, 0:4, :], wsb_r[:, 4:8, :],
                                   wsb_r[:, 8:12, :])

            # PE warmup: dummy matmuls with no DMA dependency keep the PE
            # busy (and its p-state ramping) while the first loads land.
            wu = pp.tile([128, 576], F16, tag="wu", name="wu")
            nc.vector.memset(wu[:].opt().bitcast(mybir.dt.uint32), 0)
            for n in range(14):
                ps = ps_pr.tile([128, 512], F32, tag="ps_pr", name=f"wu{n}")
                mm(nc, ps[0:64, :], wu[:, 0:64], wu[:, 64:576],
                   start=True, stop=True)

            # ones columns for the softmax denominator (V copies overwrite
            # cols 0:64 of each slot; col 64 stays at the memset value)
            nc.vector.memset(vaug[:].opt().bitcast(mybir.dt.uint32),
                             0x3C003C00)  # fp16 1.0 pair
            nc.vector.memset(vhir[:].opt().bitcast(mybir.dt.uint32),
                             0x3F800000)  # f32 1.0
            nc.vector.memset(vlor[:].opt().bitcast(mybir.dt.uint32), 0)

            def load_q(lane, c0, c1):
                lane.dma_start(out=xqT[:, :, c0:c1], in_=xqh[:, :, c0:c1])

            def load_kv(lane, p):
                sl = slice(p * 256, (p + 1) * 256)
                lane.dma_start(out=xkvT[:, :, sl], in_=xkvh[:, :, sl])

            # Dual-lane loads in compute order.  SP issues through HWDGE,
            # gpsimd through SWDGE - the two DGE paths run concurrently.
            sp, gp = nc.sync, nc.gpsimd
            gp.dma_start(out=cmask_h[:], in_=cm_h[:])
            sp.dma_start(out=wsb_h[:], in_=w_h[:])
            load_q(sp, 0, 512)                     # q1
            # k-part of pair 0 first: it gates the first S matmul
            sp.dma_start(out=xkvT[:, 0:4, 0:256], in_=xkvh[:, 0:4, 0:256])
            sp.dma_start(out=xkvT[:, 4:8, 0:256], in_=xkvh[:, 4:8, 0:256])
            load_kv(sp, 1)
            load_q(sp, 512, 1024)                  # q2
            load_kv(sp, 2)
            load_q(sp, 1024, 1536)                 # q3
            load_kv(sp, 3)
            load_q(sp, 1536, 2048)                 # q4
            load_kv(sp, 4)
            load_q(sp, 2048, 2560)                 # q5
            load_kv(sp, 5)
            load_q(sp, 2560, 3072)                 # q6
            load_kv(sp, 6)
            # (compute order: q_stage(j) precedes the k/v stages it overlaps)
            sp.dma_start(out=wsb_r[:], in_=w_r[:])
            sp.dma_start(out=xkvTr[:], in_=xkv_r[:])
            nc.vector.tensor_copy(cmask_r[:], cmask_h[:])
            sp.dma_start(out=xqTr[:], in_=xq_r[:])
            load_kv(sp, 7)
            load_q(sp, 3072, 3584)                 # q7

            def corner_projs():
                ps = ps_pr.tile([128, 512], F32, tag="ps_pr", name="pp_kr")
                for dt in range(4):
                    mm(nc, ps[0:64, 0:256], wk_rs[:, dt, :],
                       xkvTr[:, dt, :], start=(dt == 0), stop=(dt == 3))
                nc.vector.tensor_copy(ktr[:], ps[0:64, 0:256])
                for l in range(2):
                    ps = ps_pr.tile([128, 512], F32, tag="ps_pr",
                                    name=f"pp_vr{l}")
                    for dt in range(4):
                        mm(nc, ps[:, 0:D_OUT],
                           xkvTr[:, 4 + dt, l * 128:(l + 1) * 128],
                           wv_rs[:, dt, :],
                           start=(dt == 0), stop=(dt == 3))
                    nc.vector.tensor_copy(vhir[:, l, 0:D_OUT], ps[:, 0:D_OUT])
                    nc.vector.tensor_sub(
                        vlor[:, l, 0:D_OUT], ps[:, 0:D_OUT],
                        vhir[:, l, 0:D_OUT])
                ps = ps_pr.tile([128, 512], F32, tag="ps_pr", name="pp_qr")
                for dt in range(4):
                    mm(nc, ps[0:64, :], wq_rs[:, dt, :], xqTr[:, dt, :],
                       start=(dt == 0), stop=(dt == 3))
                nc.vector.tensor_copy(qtr[:], ps[0:64, :])

            def corner_rest():
                ps = ps_s.tile([128, 1024], F32, tag="ps_s", name="st_r")
                mm(nc, ps[:, 0:512], ktr[:, 0:128], qtr[:],
                   start=True, stop=True)
                mm(nc, ps[:, 512:768], ktr[:, 128:256], qtr[:, 256:512],
                   start=True, stop=True)
                nc.scalar.activation(etr[:], ps[:, 0:768], EXP)
                nc.vector.tensor_mul(etr[:], etr[:], cmask_r[:])
                pv = ps_pv.tile([128, 512], F32, tag="ps_pv", name="pv_r")
                mm(nc, pv[0:65, :], vhir[:, 0, :], etr[:, 0:512],
                   start=True, stop=False)
                mm(nc, pv[0:65, :], vlor[:, 0, :], etr[:, 0:512],
                   start=False, stop=False)
                mm(nc, pv[0:65, 256:512], vhir[:, 1, :], etr[:, 512:768],
                   start=False, stop=False)
                mm(nc, pv[0:65, 256:512], vlor[:, 1, :], etr[:, 512:768],
                   start=False, stop=True)
                nc.vector.tensor_copy(oc[:], pv[0:65, :])
                nc.sync.dma_start(out=pvt0[:], in_=oc[:])

            # ---------------- fp16 bulk ----------------
            def kproj(p):
                # k-pair p: tiles 2p, 2p+1 (cols 256p : 256p+256)
                sl = slice(p * 256, (p + 1) * 256)
                ps = ps_pr.tile([128, 512], F32, tag="ps_pr", name=f"pp_k{p}")
                for dt in range(4):
                    mm(nc, ps[0:64, 0:256], wk_h[:, dt, :],
                       xkvT[:, dt, sl], start=(dt == 0), stop=(dt == 3))
                nc.vector.tensor_copy(kt[:, sl], ps[0:64, 0:256])

            def vproj(p):
                for l in (2 * p, 2 * p + 1):
                    ps = ps_pr.tile([128, 512], F32, tag="ps_pr", name=f"pp_v{l}")
                    for dt in range(4):
                        mm(nc, ps[:, 0:D_OUT],
                           xkvT[:, 4 + dt, l * 128:(l + 1) * 128],
                           wv_h[:, dt, :],
                           start=(dt == 0), stop=(dt == 3))
                    nc.vector.tensor_copy(vaug[:, l, 0:D_OUT], ps[:, 0:D_OUT])

            def q_stage(j):
                sl = slice((j - 1) * 512, j * 512)
                ps = ps_pr.tile([128, 512], F32, tag="ps_pr", name=f"pp_q{j}")
                for dt in range(4):
                    mm(nc, ps[0:64, :], wq_h[:, dt, :], xqT[:, dt, sl],
                       start=(dt == 0), stop=(dt == 3))
                nc.vector.tensor_copy(qt[:, sl], ps[0:64, :])

            def chunk(j, diag_first=False, hooks=None):
                qs = qt[:, (j - 1) * 512:j * 512]
                pv = ps_pv.tile([128, 512], F32, tag="ps_pv", name=f"pv{j}")
                ets = {}
                # diag-first shortens the exp->mask->PV->copy tail of the
                # final chunk (its last processed pair needs no mask)
                order = [j] + list(range(j)) if diag_first else \
                    list(range(j + 1))

                def s_pair(i):
                    ps = ps_s.tile([128, 1024], F32, tag="ps_s",
                                   name=f"st{j}_{i}")
                    mm(nc, ps[:, 0:512], kt[:, (2 * i) * 128:(2 * i + 1) * 128],
                       qs, start=True, stop=True)
                    if i < j:
                        mm(nc, ps[:, 512:1024],
                           kt[:, (2 * i + 1) * 128:(2 * i + 2) * 128], qs,
                           start=True, stop=True)
                        et = etp.tile([128, 1024], F16, tag="et",
                                      name=f"et{j}_{i}")
                        nc.scalar.activation(et[:], ps[:], EXP)
                    else:
                        mm(nc, ps[:, 512:768],
                           kt[:, (2 * i + 1) * 128:(2 * i + 2) * 128],
                           qs[:, 256:512], start=True, stop=True)
                        et = etp.tile([128, 1024], F16, tag="et",
                                      name=f"et{j}_{i}")
                        nc.scalar.activation(et[:, 0:768], ps[:, 0:768], EXP)
                        nc.vector.tensor_mul(
                            et[:, 0:768], et[:, 0:768], cmask_h[:])
                    ets[i] = et

                def pv_pair(i, first, last):
                    et = ets[i]
                    mms = []
                    for t in range(4):
                        for h in range(2):
                            if i == j and h == 1 and t < 2:
                                continue
                            l = 2 * i + h
                            if h == 0:
                                esl = slice(128 * t, 128 * t + 128)
                            elif i < j:
                                esl = slice(512 + 128 * t, 512 + 128 * t + 128)
                            else:
                                esl = slice(512 + 128 * (t - 2),
                                            512 + 128 * (t - 2) + 128)
                            mms.append((t, et[:, esl], vaug[:, l, 0:D_OUT + 1]))
                    for n, (t, lh, rh) in enumerate(mms):
                        mm(nc, pv[:, t * 128:t * 128 + 65], lh, rh,
                           start=(first and n == 0),
                           stop=(last and n == len(mms) - 1))

                for n, i in enumerate(order):
                    s_pair(i)
                    for f in (hooks or {}).get(n, ()):
                        f()
                    if n > 0:
                        pv_pair(order[n - 1], first=(n == 1), last=False)
                pv_pair(order[-1], first=(j == 0), last=True)
                ost = osp.tile([128, 260], F32, tag="ost", name=f"ost{j}")
                src = pv.rearrange("p (t c) -> p t c", t=4)
                dst = ost.rearrange("p (t c) -> p t c", t=4, c=65)
                nc.vector.tensor_copy(dst[:], src[:, :, 0:65])
                lane = nc.sync if j == 7 else nc.gpsimd
                lane.dma_start(
                    out=pvtl[(j - 1) * 128:j * 128, :], in_=ost[:])

            import functools
            q_stage(1)
            kproj(0)
            chunk(1, hooks={0: (functools.partial(vproj, 0),
                                functools.partial(kproj, 1),
                                functools.partial(vproj, 1))})
            for j in range(2, 7):
                q_stage(j)
                chunk(j, hooks={0: (functools.partial(kproj, j),
                                    functools.partial(vproj, j))})
            corner_projs()
            corner_rest()
            kproj(7)
            vproj(7)
            q_stage(7)
            chunk(7, diag_first=True)
    nc.compile()
    return nc


def _prep_w(w, scale=1.0):
    # [512, 64] -> [128, 4, 64]: (p, dt, e) holds W[dt*128 + p, e]
    return np.ascontiguousarray(
        (w * scale).reshape(4, 128, D_OUT).transpose(1, 0, 2).astype(np.float32))


def _dblock(xT):
    # [512, C] -> [128, 4, C]
    return np.ascontiguousarray(
        xT.reshape(4, 128, -1).transpose(1, 0, 2))


def kernel(inputs_for_keys, inputs_for_values, inputs_for_queries, WK, WV, WQ):
    xk_f = np.asarray(inputs_for_keys, np.float32)
    xv_f = np.asarray(inputs_for_values, np.float32)
    xq_f = np.asarray(inputs_for_queries, np.float32)
    wkp = _prep_w(np.asarray(WK, np.float32))
    wvp = _prep_w(np.asarray(WV, np.float32))
    wqp = _prep_w(np.asarray(WQ, np.float32), scale=1.0 / np.sqrt(np.float32(S)))
    wcat = np.concatenate([wqp, wkp, wvp], axis=1)  # [128, 12, 64]

    if "nc" not in _CACHE:
        _CACHE["nc"] = build_nc()
    nc = _CACHE["nc"]

    # cmask[p, c]       (c in 0:512):  1 if c >= p + 128*hk      (diag h=0)
    # cmask[p, 512+cc]  (cc in 0:256): 1 if cc >= p + 128*hk     (diag h=1)
    kk = np.arange(128)[:, None]
    cc512 = np.arange(512)[None, :]
    cms = []
    for hk in range(2):
        m0 = (cc512 >= kk + 128 * hk).astype(np.float32)
        cms.append(np.ascontiguousarray(
            np.concatenate([m0, m0[:, 0:256]], axis=1)))

    in_maps = []
    for c in range(N_CORES):
        b, hk = c // 2, c % 2
        xk_g = xk_f[b].reshape(S // 128, 128, D_IN)[hk::2].reshape(SK, D_IN)
        xv_g = xv_f[b].reshape(S // 128, 128, D_IN)[hk::2].reshape(SK, D_IN)
        xq_db = _dblock(xq_f[b].T)
        xkv_db = np.concatenate([_dblock(xk_g.T), _dblock(xv_g.T)], axis=1)
        in_maps.append({
            "xq_r": np.ascontiguousarray(xq_db[:, :, 0:512]),
            "xkv_r": np.ascontiguousarray(xkv_db[:, :, 0:256]),
            "xqh": np.ascontiguousarray(xq_db[:, :, 512:]).astype(np.float16),
            "xkvh": xkv_db.astype(np.float16),
            "w_r": wcat,
            "w_h": wcat.astype(np.float16),
            "cm_r": cms[hk],
            "cm_h": cms[hk].astype(np.float16),
        })

    _CACHE["in_maps"] = in_maps
    res = run_bass_kernel_spmd(nc, in_maps, core_ids=list(range(N_CORES)))
    out = np.empty((B, S, D_OUT), np.float32)
    for b in range(B):
        full = np.empty((S, D_OUT + 1), np.float32)
        for kshard in range(2):
            r = res.results[2 * b + kshard]
            part = np.empty((S, D_OUT + 1), np.float32)
            part[0:512] = r["pvt0"].T
            part[512:] = (r["pvtl"].reshape(7, 128, 4, 65)
                          .transpose(0, 2, 1, 3).reshape(3584, 65))
            if kshard == 0:
                full[:] = part
            else:
                full += part
        out[b] = full[:, 0:D_OUT] / full[:, D_OUT:D_OUT + 1]
    return out
